# revision 25
# baseline (speedup 1.0000x reference)
"""Bahdanau-attention pooling kernel for Trainium2 (8 NeuronCores).

Reference computation (per batch b):
    u    = features @ U_w + U_b          # [L, A]
    w    = hidden_state @ W_w + W_b      # [A]
    t    = tanh(u + w)                   # [L, A]
    s    = t @ A_w (+ A_b, softmax-invariant -> dropped)
    alpha = softmax(s)                   # [L]
    ctx  = alpha @ features              # [E]

Sharding: data-parallel over batch B=128 across 8 cores (16 batches/core),
weights replicated.

Per-core dataflow (all heavy compute in bf16 on TensorE), processed in
batch PAIRS so matmul moving operands are N=392 wide:
  1. gpsimd (SWDGE) casting DMA: features f32 DRAM -> bf16 SBUF natural
     [l, (j, e)] pair tiles; one DMA per l-tile covers both batches.
  2. HWDGE xbar DMA transpose (bf16): one call per l-tile ->
     ft [128p, 2j, 16c, 208l] with e = 128c + p; U_w is loaded
     chunk-major ("(c p) a -> p c a") to match.
  3. u matmuls: 4 a-chunks x 16 e-slices, rhs [128, 2, 196], psum
     [128, 2, 196]; tanh via ScalarE activation with per-partition bias
     (U_b + W_b + hidden@W_w, transposed columns per batch).
  4. scores = A_w . t via 4 accumulated matmuls -> psum [1, 2, 196];
     softmax per batch row (exp never overflows: |s| <= 22.6).  The
     softmax + alpha-transpose section is deferred one pair so the tiny
     PE transposes never stall the in-order PE stream.
  5. alpha row -> columns by PE transpose; context for groups of 4
     batches via col-tiled (tile_position) concurrent matmuls into psum
     rows 0/32/64/96, staged and written with one DMA per group; all 16
     alpha rows staged in SBUF and written with a single DMA.
"""

import sys

import numpy as np

if "/opt/trn_rl_repo" not in sys.path:
    sys.path.insert(0, "/opt/trn_rl_repo")

B, L, E, D, A = 128, 196, 2048, 512, 512
NCORES = 8
BL = B // NCORES  # batches per core
L1 = 128  # first l tile rows
L2 = L - L1  # 68
L2P = 80  # padded to mult of 16 for xbar transpose
LP = L1 + L2P  # 208 padded l in fT
EC = E // 128  # 16 e-chunks
AC = A // 128  # 4 a-chunks
DC = D // 128  # 4 d-chunks

_CACHE = {}


def _build(repeats=1):
    from concourse import bacc, masks, mybir, tile

    f32 = mybir.dt.float32
    bf16 = mybir.dt.bfloat16
    AF = mybir.ActivationFunctionType

    nc = bacc.Bacc("TRN2", target_bir_lowering=False, debug=False,
                   num_devices=NCORES)

    feat = nc.declare_dram_parameter("features", [BL, L, E], f32, isOutput=False)
    hid = nc.declare_dram_parameter("hidden_state", [BL, D], f32, isOutput=False)
    Uw = nc.declare_dram_parameter("U_w", [E, A], f32, isOutput=False)
    Ub = nc.declare_dram_parameter("U_b", [A], f32, isOutput=False)
    Ww = nc.declare_dram_parameter("W_w", [D, A], f32, isOutput=False)
    Wb = nc.declare_dram_parameter("W_b", [A], f32, isOutput=False)
    Aw = nc.declare_dram_parameter("A_w", [A], f32, isOutput=False)
    alpha_out = nc.declare_dram_parameter("alpha", [BL, L], f32, isOutput=True)
    ctx_out = nc.declare_dram_parameter("context", [BL, E], f32, isOutput=True)

    with tile.TileContext(nc) as tc:
        with tc.tile_pool(name="const", bufs=1) as const_pool:
            ident_f = const_pool.tile([128, 128], f32)
            ident_b = const_pool.tile([128, 128], bf16)
            masks.make_identity(nc, ident_f[:])
            masks.make_identity(nc, ident_b[:])

            # U in bf16, chunk-major to match the xbar transpose output:
            # U_sb[p, c, a] = U_w[128c + p, a].  Loaded in 4 a-quarters,
            # interleaved with the first feature loads (see main loop).
            U_sb = const_pool.tile([128, EC, A], bf16)

            def load_u_quarter(ac):
                nc.gpsimd.dma_start(
                    out=U_sb[:, :, 128 * ac:128 * (ac + 1)],
                    in_=Uw[:, 128 * ac:128 * (ac + 1)].rearrange(
                        "(c p) a -> p c a", c=EC))

            # W in f32, chunked: W_sb[p, dc, a] = W_w[128*dc + p, a]
            # (loaded after the small preamble DMAs below)
            W_sb = const_pool.tile([128, DC, A], f32)

            bias_col = const_pool.tile([128, AC], f32)  # [p, ac] = Ub+Wb at a=128ac+p
            A_col = const_pool.tile([128, AC], bf16)  # [p, ac] = A_w[128ac+p]
            hT_sb = const_pool.tile([128, DC, BL], f32)
            wT_sb = const_pool.tile([128, AC, BL], f32)

            with (
                tc.tile_pool(name="pre_sb", bufs=1) as pre_sb,
                tc.tile_pool(name="pre_ps", bufs=2, space="PSUM") as pre_ps,
            ):
                # bias rows [4, 128]: row c holds x[128c : 128c+128]
                ub_row = pre_sb.tile([AC, 128], f32)
                nc.sync.dma_start(
                    out=ub_row[:], in_=Ub[:].rearrange("(c p) -> c p", c=AC))
                wb_row = pre_sb.tile([AC, 128], f32)
                nc.sync.dma_start(
                    out=wb_row[:], in_=Wb[:].rearrange("(c p) -> c p", c=AC))
                aw_row = pre_sb.tile([AC, 128], f32)
                nc.sync.dma_start(
                    out=aw_row[:], in_=Aw[:].rearrange("(c p) -> c p", c=AC))

                bias_row = pre_sb.tile([AC, 128], f32)
                nc.vector.tensor_add(bias_row[:], ub_row[:], wb_row[:])
                ps_b = pre_ps.tile([128, AC], f32, tag="preps")
                nc.tensor.transpose(ps_b[:], bias_row[:], ident_f[0:AC, 0:AC])
                nc.scalar.copy(bias_col[:], ps_b[:])

                ps_a = pre_ps.tile([128, AC], f32, tag="preps")
                nc.tensor.transpose(ps_a[:], aw_row[:], ident_f[0:AC, 0:AC])
                nc.scalar.copy(A_col[:], ps_a[:])

                # hidden [16, 512] -> hT_sb[p, dc, b] = hidden[b, 128*dc + p]
                h_sb = pre_sb.tile([BL, D], f32)
                nc.sync.dma_start(out=h_sb[:], in_=hid[:])
                nc.sync.dma_start(
                    out=W_sb[:], in_=Ww[:].rearrange("(c p) a -> p c a", c=DC))
                for dc in range(DC):
                    ps_h = pre_ps.tile([128, BL], f32, tag="preps")
                    nc.tensor.transpose(
                        ps_h[:], h_sb[:, 128 * dc:128 * (dc + 1)],
                        ident_f[0:BL, 0:BL])
                    nc.scalar.copy(hT_sb[:, dc, :], ps_h[:])

                # wT_sb[p, ac, b] = (hidden[b] @ W_w)[128ac+p] + Ub[..] + Wb[..]
                for ac in range(AC):
                    ps_w = pre_ps.tile([128, BL], f32, tag="preps")
                    for dc in range(DC):
                        nc.tensor.matmul(
                            ps_w[:],
                            lhsT=W_sb[:, dc, 128 * ac:128 * (ac + 1)],
                            rhs=hT_sb[:, dc, :],
                            start=(dc == 0), stop=(dc == DC - 1))
                    nc.scalar.activation(
                        wT_sb[:, ac, :], ps_w[:], AF.Identity,
                        bias=bias_col[:, ac:ac + 1])

            with (
                tc.tile_pool(name="fn", bufs=4) as fn_pool,
                tc.tile_pool(name="ft", bufs=2) as ft_pool,
                tc.tile_pool(name="tt", bufs=8) as t_pool,
                tc.tile_pool(name="small", bufs=4) as small_pool,
                tc.tile_pool(name="outsb", bufs=1) as out_pool,
                tc.tile_pool(name="stage", bufs=2) as stage_pool,
                tc.tile_pool(name="ps_u", bufs=5, space="PSUM") as psum_u_pool,
                                tc.tile_pool(name="ps_c", bufs=3, space="PSUM") as psum_c_pool,
            ):
                # all 16 alpha rows staged on partition 0; single DMA at end
                alpha_gt = out_pool.tile([1, BL, L], f32)

                for rep in range(repeats):
                    fn1s, fn2s, acols = {}, {}, {}

                    pend = {}

                    def do_softmax(p, ps_s):
                        # softmax per batch (no max-sub: |s| <= 22.6)
                        for j in range(2):
                            b = 2 * p + j
                            e_row = small_pool.tile([1, L], f32, tag="erow",
                                                    name="e_row")
                            s_sum = small_pool.tile([1, 1], f32, tag="ssum",
                                                    name="s_sum")
                            nc.scalar.activation(
                                e_row[:], ps_s[0:1, j, :], AF.Exp,
                                accum_out=s_sum[:])
                            r_sum = small_pool.tile([1, 1], f32, tag="rsum",
                                                    name="r_sum")
                            nc.vector.reciprocal(r_sum[:], s_sum[:])
                            nc.vector.tensor_scalar_mul(
                                alpha_gt[0:1, b, :], e_row[:], r_sum[:])
                            alpha_b = small_pool.tile([1, L], bf16, tag="ab",
                                                      name="alpha_b")
                            nc.vector.tensor_copy(alpha_b[:],
                                                  alpha_gt[0:1, b, :])
                            # alpha row -> columns (PE transpose); columns 0
                            # and 2 so each psum access is 4-byte aligned
                            ps_t = psum_c_pool.tile([128, 4], bf16, tag="ctx",
                                                    name="ps_t")
                            nc.tensor.transpose(
                                ps_t[:, 0:1], alpha_b[0:1, 0:L1],
                                ident_b[0:1, 0:1])
                            nc.tensor.transpose(
                                ps_t[0:L2, 2:3], alpha_b[0:1, L1:L],
                                ident_b[0:1, 0:1])
                            acol = small_pool.tile([128, 4], bf16, tag="acol",
                                                   bufs=8, name="acol")
                            nc.vector.tensor_copy(acol[:], ps_t[:])
                            acols[b] = acol

                    def emit_ctx_group(gp, npairs=2):
                        # col-tiled concurrent matmuls, psum rows 0/32/64/96
                        g0 = 2 * gp
                        nb = 2 * npairs
                        stage = stage_pool.tile([128, E], f32, tag="stage",
                                                name="stage")
                        for c4 in range(4):
                            ps_c = psum_c_pool.tile([128, 512], f32,
                                                    tag="ctx", name="ps_c")
                            for j in range(nb):
                                b = g0 + j
                                nc.tensor.matmul(
                                    ps_c[32 * j:32 * j + 1, :],
                                    lhsT=acols[b][:, 0:1],
                                    rhs=fn1s[b][:, 512 * c4:512 * (c4 + 1)],
                                    start=True, stop=False,
                                    tile_position=(0, 32 * j))
                                nc.tensor.matmul(
                                    ps_c[32 * j:32 * j + 1, :],
                                    lhsT=acols[b][0:L2, 2:3],
                                    rhs=fn2s[b][0:L2, 512 * c4:512 * (c4 + 1)],
                                    start=False, stop=True,
                                    tile_position=(0, 32 * j))
                            # whole-tile copy: rows other than 0/32/64/96
                            # are garbage, never read
                            nc.scalar.copy(
                                stage[:, 512 * c4:512 * (c4 + 1)], ps_c[:])
                        nc.sync.dma_start(
                            out=ctx_out[g0:g0 + nb, :],
                            in_=stage[:].rearrange(
                                "(j r) n -> j r n", j=4)[0:nb, 0, :])
                    for pair in range(BL // 2):
                        b0 = 2 * pair
                        # -- load natural bf16 (casting SWDGE DMA), both
                        #    batches of the pair in one DMA; one xbar
                        #    transpose per l-tile covers both batches:
                        #    transposed row r = (j*16 + c)*128 + p --
                        ft = ft_pool.tile([128, 2, EC, LP], bf16, tag="ft")
                        fnp1 = fn_pool.tile([128, 2, E], bf16, tag="fn1")
                        nc.gpsimd.dma_start(
                            out=fnp1[:],
                            in_=feat[b0:b0 + 2, 0:L1, :].rearrange(
                                "j l e -> l j e"))
                        if rep == 0 and pair == 0:
                            load_u_quarter(0)
                        fnp2 = fn_pool.tile([L2P, 2, E], bf16, tag="fn2")
                        nc.gpsimd.dma_start(
                            out=fnp2[0:L2, :, :],
                            in_=feat[b0:b0 + 2, L1:L, :].rearrange(
                                "j l e -> l j e"))
                        nc.sync.dma_start(
                            out=ft[:, :, :, 0:L1],
                            in_=fnp1[:].rearrange("p j e -> p (j e)"),
                            transpose=True)
                        nc.sync.dma_start(
                            out=ft[:, :, :, L1:LP],
                            in_=fnp2[:].rearrange("p j e -> p (j e)"),
                            transpose=True)
                        for j in range(2):
                            b = b0 + j
                            fn1s[b] = fnp1[:, j, :]
                            fn2s[b] = fnp2[:, j, :]

                        if rep == 0 and pair == 0:
                            for q in range(1, 4):
                                load_u_quarter(q)

                        # -- u matmuls (both batches per MM, N=392) + tanh --
                        tts = []
                        for ac in range(AC):
                            ps_u = psum_u_pool.tile([128, 2, L], f32, tag="psu")
                            for c in range(EC):
                                nc.tensor.matmul(
                                    ps_u[:],
                                    lhsT=U_sb[:, c, 128 * ac:128 * (ac + 1)],
                                    rhs=ft[:, :, c, 0:L],
                                    start=(c == 0), stop=(c == EC - 1))
                            tt = t_pool.tile([128, 2, L], bf16, tag="t")
                            for j in range(2):
                                nc.scalar.activation(
                                    tt[:, j, :], ps_u[:, j, :], AF.Tanh,
                                    bias=wT_sb[:, ac, b0 + j:b0 + j + 1])
                            tts.append(tt)

                        # -- scores rows [1, 2, L] --
                        ps_s = psum_c_pool.tile([1, 2, L], f32, tag="ctx")
                        for ac in range(AC):
                            nc.tensor.matmul(
                                ps_s[:], lhsT=A_col[:, ac:ac + 1], rhs=tts[ac][:],
                                start=(ac == 0), stop=(ac == AC - 1))

                        # -- softmax + alpha transpose, deferred one pair
                        #    so the tiny PE transposes never stall the PE
                        #    stream; ctx bursts deferred accordingly --
                        pend[pair] = ps_s
                        if pair >= 1:
                            do_softmax(pair - 1, pend.pop(pair - 1))
                        if pair >= 2 and pair % 2 == 0:
                            emit_ctx_group(pair - 2)

                    emit_ctx_group(BL // 2 - 2, npairs=1)
                    do_softmax(BL // 2 - 1, pend.pop(BL // 2 - 1))
                    emit_ctx_group(BL // 2 - 1, npairs=1)
                    nc.sync.dma_start(
                        out=alpha_out[:], in_=alpha_gt[0:1, :, :])

    nc.finalize()
    return nc


def _get_nc(repeats=1):
    key = ("nc", repeats)
    if key not in _CACHE:
        _CACHE[key] = _build(repeats)
    return _CACHE[key]


def kernel(**inputs):
    from concourse.bass_utils import run_bass_kernel_spmd

    feats = np.ascontiguousarray(np.asarray(inputs["features"], np.float32))
    hs = np.ascontiguousarray(np.asarray(inputs["hidden_state"], np.float32))
    shared = {
        "U_w": np.ascontiguousarray(np.asarray(inputs["U_w"], np.float32)),
        "U_b": np.ascontiguousarray(np.asarray(inputs["U_b"], np.float32)),
        "W_w": np.ascontiguousarray(np.asarray(inputs["W_w"], np.float32)),
        "W_b": np.ascontiguousarray(np.asarray(inputs["W_b"], np.float32)),
        "A_w": np.ascontiguousarray(np.asarray(inputs["A_w"], np.float32)),
    }

    nc = _get_nc()
    in_maps = []
    for i in range(NCORES):
        m = dict(shared)
        m["features"] = feats[i * BL:(i + 1) * BL]
        m["hidden_state"] = hs[i * BL:(i + 1) * BL]
        in_maps.append(m)

    res = run_bass_kernel_spmd(nc, in_maps, core_ids=list(range(NCORES)))
    alpha = np.concatenate(
        [np.asarray(res.results[i]["alpha"]) for i in range(NCORES)], axis=0)
    ctx = np.concatenate(
        [np.asarray(res.results[i]["context"]) for i in range(NCORES)], axis=0)
    return alpha.astype(np.float32), ctx.astype(np.float32)


# revision 26
# speedup vs baseline: 1.0068x; 1.0068x over previous
"""Bahdanau-attention pooling kernel for Trainium2 (8 NeuronCores).

Reference computation (per batch b):
    u    = features @ U_w + U_b          # [L, A]
    w    = hidden_state @ W_w + W_b      # [A]
    t    = tanh(u + w)                   # [L, A]
    s    = t @ A_w (+ A_b, softmax-invariant -> dropped)
    alpha = softmax(s)                   # [L]
    ctx  = alpha @ features              # [E]

Sharding: data-parallel over batch B=128 across 8 cores (16 batches/core),
weights replicated.

Per-core dataflow (all heavy compute in bf16 on TensorE):
  1. gpsimd (SWDGE) casting DMA: features f32 DRAM -> bf16 SBUF, natural
     layout [l, e] tiles ([128,2048] + [68(->80),2048]).
  2. HWDGE xbar DMA transpose (bf16): natural -> fT [128p, 16c, 208l],
     e = 16*p + c.  U_w is loaded pre-permuted to the same e-order so the
     contraction matches.
  3. u matmuls: 4 a-chunks x 16 e-slices, psum [128, 196];
     tanh via ScalarE activation with per-partition bias
     (U_b + W_b + hidden@W_w, transposed column per batch).
  4. scores = A_w . t via 4 accumulated matmuls -> psum row [1, 196];
     softmax on the row (exp never overflows: |s| <= 22.6).
  5. alpha row -> column by PE transpose; context = alpha.T @ f_natural
     via 8 accumulated matmuls -> [1, 2048].
"""

import sys

import numpy as np

if "/opt/trn_rl_repo" not in sys.path:
    sys.path.insert(0, "/opt/trn_rl_repo")

B, L, E, D, A = 128, 196, 2048, 512, 512
NCORES = 8
BL = B // NCORES  # batches per core
L1 = 128  # first l tile rows
L2 = L - L1  # 68
L2P = 80  # padded to mult of 16 for xbar transpose
LP = L1 + L2P  # 208 padded l in fT
EC = E // 128  # 16 e-chunks
AC = A // 128  # 4 a-chunks
DC = D // 128  # 4 d-chunks

_CACHE = {}


def _build(repeats=1):
    from concourse import bacc, masks, mybir, tile

    f32 = mybir.dt.float32
    bf16 = mybir.dt.bfloat16
    AF = mybir.ActivationFunctionType

    nc = bacc.Bacc("TRN2", target_bir_lowering=False, debug=False,
                   num_devices=NCORES)

    feat = nc.declare_dram_parameter("features", [BL, L, E], f32, isOutput=False)
    hid = nc.declare_dram_parameter("hidden_state", [BL, D], f32, isOutput=False)
    Uw = nc.declare_dram_parameter("U_w", [E, A], f32, isOutput=False)
    Ub = nc.declare_dram_parameter("U_b", [A], f32, isOutput=False)
    Ww = nc.declare_dram_parameter("W_w", [D, A], f32, isOutput=False)
    Wb = nc.declare_dram_parameter("W_b", [A], f32, isOutput=False)
    Aw = nc.declare_dram_parameter("A_w", [A], f32, isOutput=False)
    alpha_out = nc.declare_dram_parameter("alpha", [BL, L], f32, isOutput=True)
    ctx_out = nc.declare_dram_parameter("context", [BL, E], f32, isOutput=True)

    with tile.TileContext(nc) as tc:
        with tc.tile_pool(name="const", bufs=1) as const_pool:
            ident_f = const_pool.tile([128, 128], f32)
            ident_b = const_pool.tile([128, 128], bf16)
            masks.make_identity(nc, ident_f[:])
            masks.make_identity(nc, ident_b[:])

            # U in bf16, chunk-major to match the xbar transpose output:
            # U_sb[p, c, a] = U_w[128c + p, a].  Loaded in 4 a-quarters,
            # interleaved with the first feature loads (see main loop).
            U_sb = const_pool.tile([128, EC, A], bf16)

            def load_u_quarter(ac):
                nc.gpsimd.dma_start(
                    out=U_sb[:, :, 128 * ac:128 * (ac + 1)],
                    in_=Uw[:, 128 * ac:128 * (ac + 1)].rearrange(
                        "(c p) a -> p c a", c=EC))

            # W in f32, chunked: W_sb[p, dc, a] = W_w[128*dc + p, a]
            # (loaded after the small preamble DMAs below)
            W_sb = const_pool.tile([128, DC, A], f32)

            bias_col = const_pool.tile([128, AC], f32)  # [p, ac] = Ub+Wb at a=128ac+p
            A_col = const_pool.tile([128, AC], bf16)  # [p, ac] = A_w[128ac+p]
            hT_sb = const_pool.tile([128, DC, BL], f32)
            wT_sb = const_pool.tile([128, AC, BL], f32)

            with (
                tc.tile_pool(name="pre_sb", bufs=1) as pre_sb,
                tc.tile_pool(name="pre_ps", bufs=2, space="PSUM") as pre_ps,
            ):
                # bias rows [4, 128]: row c holds x[128c : 128c+128]
                ub_row = pre_sb.tile([AC, 128], f32)
                nc.sync.dma_start(
                    out=ub_row[:], in_=Ub[:].rearrange("(c p) -> c p", c=AC))
                wb_row = pre_sb.tile([AC, 128], f32)
                nc.sync.dma_start(
                    out=wb_row[:], in_=Wb[:].rearrange("(c p) -> c p", c=AC))
                aw_row = pre_sb.tile([AC, 128], f32)
                nc.sync.dma_start(
                    out=aw_row[:], in_=Aw[:].rearrange("(c p) -> c p", c=AC))

                bias_row = pre_sb.tile([AC, 128], f32)
                nc.vector.tensor_add(bias_row[:], ub_row[:], wb_row[:])
                ps_b = pre_ps.tile([128, AC], f32, tag="preps")
                nc.tensor.transpose(ps_b[:], bias_row[:], ident_f[0:AC, 0:AC])
                nc.scalar.copy(bias_col[:], ps_b[:])

                ps_a = pre_ps.tile([128, AC], f32, tag="preps")
                nc.tensor.transpose(ps_a[:], aw_row[:], ident_f[0:AC, 0:AC])
                nc.scalar.copy(A_col[:], ps_a[:])

                # hidden [16, 512] -> hT_sb[p, dc, b] = hidden[b, 128*dc + p]
                h_sb = pre_sb.tile([BL, D], f32)
                nc.sync.dma_start(out=h_sb[:], in_=hid[:])
                nc.sync.dma_start(
                    out=W_sb[:], in_=Ww[:].rearrange("(c p) a -> p c a", c=DC))
                for dc in range(DC):
                    ps_h = pre_ps.tile([128, BL], f32, tag="preps")
                    nc.tensor.transpose(
                        ps_h[:], h_sb[:, 128 * dc:128 * (dc + 1)],
                        ident_f[0:BL, 0:BL])
                    nc.scalar.copy(hT_sb[:, dc, :], ps_h[:])

                # wT_sb[p, ac, b] = (hidden[b] @ W_w)[128ac+p] + Ub[..] + Wb[..]
                for ac in range(AC):
                    ps_w = pre_ps.tile([128, BL], f32, tag="preps")
                    for dc in range(DC):
                        nc.tensor.matmul(
                            ps_w[:],
                            lhsT=W_sb[:, dc, 128 * ac:128 * (ac + 1)],
                            rhs=hT_sb[:, dc, :],
                            start=(dc == 0), stop=(dc == DC - 1))
                    nc.scalar.activation(
                        wT_sb[:, ac, :], ps_w[:], AF.Identity,
                        bias=bias_col[:, ac:ac + 1])

            with (
                tc.tile_pool(name="fn", bufs=4) as fn_pool,
                tc.tile_pool(name="ft", bufs=2) as ft_pool,
                tc.tile_pool(name="tt", bufs=8) as t_pool,
                tc.tile_pool(name="small", bufs=4) as small_pool,
                tc.tile_pool(name="outsb", bufs=1) as out_pool,
                tc.tile_pool(name="stage", bufs=2) as stage_pool,
                tc.tile_pool(name="ps_u", bufs=5, space="PSUM") as psum_u_pool,
                                tc.tile_pool(name="ps_c", bufs=3, space="PSUM") as psum_c_pool,
            ):
                # all 16 alpha rows staged on partition 0; single DMA at end
                alpha_gt = out_pool.tile([1, BL, L], f32)

                for rep in range(repeats):
                    fn1s, fn2s, acols = {}, {}, {}

                    pend = {}

                    def do_softmax(p, ps_s):
                        # softmax per batch (no max-sub: |s| <= 22.6)
                        for j in range(2):
                            b = 2 * p + j
                            e_row = small_pool.tile([1, L], f32, tag="erow",
                                                    name="e_row")
                            s_sum = small_pool.tile([1, 1], f32, tag="ssum",
                                                    name="s_sum")
                            nc.scalar.activation(
                                e_row[:], ps_s[0:1, j, :], AF.Exp,
                                accum_out=s_sum[:])
                            r_sum = small_pool.tile([1, 1], f32, tag="rsum",
                                                    name="r_sum")
                            nc.vector.reciprocal(r_sum[:], s_sum[:])
                            nc.vector.tensor_scalar_mul(
                                alpha_gt[0:1, b, :], e_row[:], r_sum[:])
                            alpha_b = small_pool.tile([1, L], bf16, tag="ab",
                                                      name="alpha_b")
                            nc.vector.tensor_copy(alpha_b[:],
                                                  alpha_gt[0:1, b, :])
                            # alpha row -> columns (PE transpose); columns 0
                            # and 2 so each psum access is 4-byte aligned
                            ps_t = psum_c_pool.tile([128, 4], bf16, tag="ctx",
                                                    name="ps_t")
                            nc.tensor.transpose(
                                ps_t[:, 0:1], alpha_b[0:1, 0:L1],
                                ident_b[0:1, 0:1])
                            nc.tensor.transpose(
                                ps_t[0:L2, 2:3], alpha_b[0:1, L1:L],
                                ident_b[0:1, 0:1])
                            acol = small_pool.tile([128, 4], bf16, tag="acol",
                                                   bufs=8, name="acol")
                            nc.vector.tensor_copy(acol[:], ps_t[:])
                            acols[b] = acol

                    def emit_ctx_group(gp, npairs=2):
                        # col-tiled concurrent matmuls, psum rows 0/32/64/96
                        g0 = 2 * gp
                        nb = 2 * npairs
                        stage = stage_pool.tile([128, E], f32, tag="stage",
                                                name="stage")
                        for c4 in range(4):
                            ps_c = psum_c_pool.tile([128, 512], f32,
                                                    tag="ctx", name="ps_c")
                            for j in range(nb):
                                b = g0 + j
                                nc.tensor.matmul(
                                    ps_c[32 * j:32 * j + 1, :],
                                    lhsT=acols[b][:, 0:1],
                                    rhs=fn1s[b][:, 512 * c4:512 * (c4 + 1)],
                                    start=True, stop=False,
                                    tile_position=(0, 32 * j))
                                nc.tensor.matmul(
                                    ps_c[32 * j:32 * j + 1, :],
                                    lhsT=acols[b][0:L2, 2:3],
                                    rhs=fn2s[b][0:L2, 512 * c4:512 * (c4 + 1)],
                                    start=False, stop=True,
                                    tile_position=(0, 32 * j))
                            # whole-tile copy: rows other than 0/32/64/96
                            # are garbage, never read
                            nc.scalar.copy(
                                stage[:, 512 * c4:512 * (c4 + 1)], ps_c[:])
                        nc.sync.dma_start(
                            out=ctx_out[g0:g0 + nb, :],
                            in_=stage[:].rearrange(
                                "(j r) n -> j r n", j=4)[0:nb, 0, :])
                    for pair in range(BL // 2):
                        b0 = 2 * pair
                        # -- load natural bf16 (casting SWDGE DMA), both
                        #    batches of the pair in one DMA; one xbar
                        #    transpose per l-tile covers both batches:
                        #    transposed row r = (j*16 + c)*128 + p --
                        ft = ft_pool.tile([128, 2, EC, LP], bf16, tag="ft")
                        fnp1 = fn_pool.tile([128, 2, E], bf16, tag="fn1")
                        nc.gpsimd.dma_start(
                            out=fnp1[:],
                            in_=feat[b0:b0 + 2, 0:L1, :].rearrange(
                                "j l e -> l j e"))
                        if rep == 0 and pair == 0:
                            load_u_quarter(0)
                        fnp2 = fn_pool.tile([L2P, 2, E], bf16, tag="fn2")
                        nc.gpsimd.dma_start(
                            out=fnp2[0:L2, :, :],
                            in_=feat[b0:b0 + 2, L1:L, :].rearrange(
                                "j l e -> l j e"))
                        nc.sync.dma_start(
                            out=ft[:, :, :, 0:L1],
                            in_=fnp1[:].rearrange("p j e -> p (j e)"),
                            transpose=True)
                        nc.sync.dma_start(
                            out=ft[:, :, :, L1:LP],
                            in_=fnp2[:].rearrange("p j e -> p (j e)"),
                            transpose=True)
                        for j in range(2):
                            b = b0 + j
                            fn1s[b] = fnp1[:, j, :]
                            fn2s[b] = fnp2[:, j, :]

                        if rep == 0 and pair == 0:
                            for q in range(1, 4):
                                load_u_quarter(q)

                        # -- u matmuls (both batches per MM, N=392) + tanh --
                        tts = []
                        for ac in range(AC):
                            ps_u = psum_u_pool.tile([128, 2, L], f32, tag="psu")
                            for c in range(EC):
                                nc.tensor.matmul(
                                    ps_u[:],
                                    lhsT=U_sb[:, c, 128 * ac:128 * (ac + 1)],
                                    rhs=ft[:, :, c, 0:L],
                                    start=(c == 0), stop=(c == EC - 1))
                            tt = t_pool.tile([128, 2, L], bf16, tag="t")
                            for j in range(2):
                                nc.scalar.activation(
                                    tt[:, j, :], ps_u[:, j, :], AF.Tanh,
                                    bias=wT_sb[:, ac, b0 + j:b0 + j + 1])
                            tts.append(tt)

                        # -- scores rows [1, 2, L] --
                        ps_s = psum_c_pool.tile([1, 2, L], f32, tag="ctx")
                        for ac in range(AC):
                            nc.tensor.matmul(
                                ps_s[:], lhsT=A_col[:, ac:ac + 1], rhs=tts[ac][:],
                                start=(ac == 0), stop=(ac == AC - 1))

                        # -- softmax + alpha transpose, deferred one pair
                        #    so the tiny PE transposes never stall the PE
                        #    stream; ctx bursts deferred accordingly --
                        pend[pair] = ps_s
                        if pair >= 1:
                            do_softmax(pair - 1, pend.pop(pair - 1))
                        if pair >= 2 and pair % 2 == 0:
                            emit_ctx_group(pair - 2)

                    emit_ctx_group(BL // 2 - 2, npairs=1)
                    do_softmax(BL // 2 - 1, pend.pop(BL // 2 - 1))
                    emit_ctx_group(BL // 2 - 1, npairs=1)
                    nc.sync.dma_start(
                        out=alpha_out[:], in_=alpha_gt[0:1, :, :])

    nc.finalize()
    return nc


def _get_nc(repeats=1):
    key = ("nc", repeats)
    if key not in _CACHE:
        _CACHE[key] = _build(repeats)
    return _CACHE[key]


def kernel(**inputs):
    from concourse.bass_utils import run_bass_kernel_spmd

    feats = np.ascontiguousarray(np.asarray(inputs["features"], np.float32))
    hs = np.ascontiguousarray(np.asarray(inputs["hidden_state"], np.float32))
    shared = {
        "U_w": np.ascontiguousarray(np.asarray(inputs["U_w"], np.float32)),
        "U_b": np.ascontiguousarray(np.asarray(inputs["U_b"], np.float32)),
        "W_w": np.ascontiguousarray(np.asarray(inputs["W_w"], np.float32)),
        "W_b": np.ascontiguousarray(np.asarray(inputs["W_b"], np.float32)),
        "A_w": np.ascontiguousarray(np.asarray(inputs["A_w"], np.float32)),
    }

    nc = _get_nc()
    in_maps = []
    for i in range(NCORES):
        m = dict(shared)
        m["features"] = feats[i * BL:(i + 1) * BL]
        m["hidden_state"] = hs[i * BL:(i + 1) * BL]
        in_maps.append(m)

    res = run_bass_kernel_spmd(nc, in_maps, core_ids=list(range(NCORES)))
    alpha = np.concatenate(
        [np.asarray(res.results[i]["alpha"]) for i in range(NCORES)], axis=0)
    ctx = np.concatenate(
        [np.asarray(res.results[i]["context"]) for i in range(NCORES)], axis=0)
    return alpha.astype(np.float32), ctx.astype(np.float32)
